# revision 22
# baseline (speedup 1.0000x reference)
"""BuildCostVolume kernel for 8 Trainium2 NeuronCores.

Decomposition: the 9 strided dilated convs (disparities d=-4..4) reduce to
729 taps (d,u,v): cost[b,co,d,h,w] = sum_{ci,u,v} Wd[d][co,ci,u,v] *
X[b,ci,u,v, h+d(4-u), w+d(4-v)] where X is the view-decomposed light field
(X[b,ci,u,v,h,w] = x[b,ci,9h+u,9w+v], zero outside) and Wd flips (u,v) for
d>0.  Each tap is a K=64(ci) x M=64(co) matmul over spatial positions.

Sharding: core = (batch b, h-half).  Each core holds all 81 views' h-windows
(zero-padded to uniform per-u heights) resident in SBUF as 41 view-pair
tiles ([128, R*48] bf16: two views stacked on partition halves).  Taps run
as 4-way concurrent matmuls via tile_position (2 row-groups x 2 col-groups),
accumulating per (d, 8-row subchunk) into PSUM tiles keyed (sub, rh) with
the col-group picking the partition half.  Evac: the Activation engine
copies one psum quadrant-partial while DVE does the 3 accumulating adds
(cross-partition reads are only legal from PSUM; one PSUM operand per op)
— moving the copy off DVE took the measured sweep from ~136us to ~112us.
Input load ships only the valid rows of each view block (79% of bytes; the
load is HBM-bound across the 8 cores) and branches on core id (tc.If on
nc.partition_id(), which loads the condition on ALL engines — an SP-only
register deadlocks the DVE memsets) to zero the class-dependent pad rows
on the otherwise-idle DVE while the DMAs stream.  Weights are shipped once
as the 81 pre-transposed taps [ci, co] (10KB/partition, resident in SBUF;
lhsT APs index them by (kh,kw) per (d,tap)) instead of a per-d wpack —
every disparity reuses the same 81 slices, so the old layout shipped the
weight bytes 4.5x over and spent 9 DMA sync points per sweep.

Measured dead ends (see _transcript/memory): fp8e4 DoubleRow is blocked by
the s3d3_mm_valid_dst_partition ISA check at col position 64 (DoubleRow's
128-wide stationary uses all PE columns -> only 2-way row tiling), and the
3-term error compensation needed for rel_err<2e-2 makes it net slower than
bf16.  A ch-fixed group restructure (2 psum partials/group, 4-group sets)
measured ~163us/sweep — worse than this layout.
"""

import numpy as np
import ml_dtypes

A = 9           # angular resolution
H = 48          # spatial h/w per view
C = 64          # channels (ci = co = 64)
B = 4           # batch
ND = 9          # disparities -4..4
HH = 24         # h rows per core (half)
SUB = 8         # output h rows per psum accumulation group
NSUB = HH // SUB
NSLOT = 41      # weight slots per row-half per d
N_CORES = 8

BF16 = ml_dtypes.bfloat16


def _geometry():
    """Static tap/tile geometry shared by host packing and device program."""
    pairs = []            # (viewA, viewB-or-None, R)
    # seed tiles first: their 4 views are the full-width (v=4) taps that
    # carry start=True per tile position, so their DMAs must land first.
    pairs.append(((0, 4), (8, 4), HH + 8 * 4))
    pairs.append(((1, 4), (7, 4), HH + 8 * 3))
    for v in range(A):
        for u in range(4):
            if v == 4 and u in (0, 1):
                continue
            pairs.append(((u, v), (8 - u, v), HH + 8 * (4 - u)))
    for k in range(4):
        pairs.append(((4, k), (4, k + 5), HH))
    pairs.append(((4, 4), None, HH))

    view_loc = {}
    offs = []
    off = 0
    for j, (va, vb, R) in enumerate(pairs):
        view_loc[va] = (j, 0)
        if vb is not None:
            view_loc[vb] = (j, 1)
        offs.append(off)
        off += R * H
    F = off

    # one tap order for every d: tile order (= DMA arrival order), seeds
    # first covering the 4 (rh, ch) positions, then ch alternating per rh
    # so consecutive taps hit different tile positions.
    taps = []
    ch_ctr = {0: 0, 1: 0}
    slot_ctr = {0: 0, 1: 0}
    for j, (va, vb, R) in enumerate(pairs):
        for half, view in ((0, va), (1, vb)):
            if view is None:
                continue
            u, v = view
            rh = half
            ch = ch_ctr[rh] % 2
            ch_ctr[rh] += 1
            s = slot_ctr[rh]
            slot_ctr[rh] += 1
            taps.append((u, v, rh, ch, s))
    assert len(taps) == 81
    # seeds sanity: first 4 taps cover all 4 positions with v=4 views
    seed_pos = {(rh, ch) for (u, v, rh, ch, s) in taps[:4]}
    assert len(seed_pos) == 4 and all(v == 4 for (u, v, _, _, _) in taps[:4])

    return pairs, view_loc, offs, F, taps


_PAIRS, _VIEW_LOC, _OFFS, _F, _TAPS1 = _geometry()
_TAPS = [_TAPS1] * ND  # same order for every d (kept for prepare_inputs)
_NC_CACHE = {}


def _build_nc(repeat=1):
    import os
    import concourse.bacc as bacc
    import concourse.mybir as mybir
    import concourse.tile as tile

    variant = os.environ.get("KVARIANT", "full")  # timing experiments only

    nc = bacc.Bacc(None, target_bir_lowering=False)
    xwin_d = nc.dram_tensor("xwin", [128, _F], mybir.dt.bfloat16,
                            kind="ExternalInput")
    # 81 pre-transposed W taps [ci, co], duplicated on both partition
    # halves: every (d, tap) weight block is one of these slices, so no
    # per-d wpack staging (saves 5.3MB/core of the HBM-bound input and 9
    # DMA sync points per sweep).
    wt_d = nc.dram_tensor("wt", [128, A * A * C], mybir.dt.bfloat16,
                          kind="ExternalInput")
    out_d = nc.dram_tensor("out", [C, ND * NSUB * SUB * H], mybir.dt.float32,
                           kind="ExternalOutput")

    with tile.TileContext(nc) as tc:
        with tc.tile_pool(name="xw", bufs=1) as xpool, \
             tc.tile_pool(name="wp", bufs=1) as wpool, \
             tc.tile_pool(name="ps", bufs=2, space="PSUM") as ppool, \
             tc.tile_pool(name="ob", bufs=4) as opool:

            # resident weight taps; DMA first so the seed MMs aren't blocked
            wtr = wpool.tile([128, A * A * C], mybir.dt.bfloat16, tag="wtr")
            nc.sync.dma_start(out=wtr[:], in_=wt_d[:])

            xtiles = []
            xviews = []
            for j, (va, vb, R) in enumerate(_PAIRS):
                t = xpool.tile([128, R * H], mybir.dt.bfloat16, tag=f"x{j}")
                xtiles.append(t)
                # h-major [p, r, c]: contiguous 48-elem inner runs stream at
                # full PE rate (short strided inner runs are ~2.5x slower)
                xviews.append(t[:].rearrange("p (r c) -> p r c", r=R, c=H))

            def load_x():
                # Valid-rows-only input: each view block [R=24+8au, 48] has
                # 4au zero rows (top for h-half 0 cores, bottom for h-half 1)
                # that the host packs but we never ship — DVE memsets them
                # while the DMAs stream the remaining 79%.  Both views of a
                # pair share au, so pad ranges are tile-uniform.  The pad
                # side depends on the core's h-half -> branch on core id.
                pid = nc.partition_id()
                for cls in (0, 1):
                    with tc.If(pid % 2 == cls):
                        for j, (va, vb, R) in enumerate(_PAIRS):
                            pad = ((R - HH) // 8) * 4  # 4*au
                            s, e = (pad, R) if cls == 0 else (0, R - pad)
                            if pad:
                                z0, z1 = ((0, pad) if cls == 0
                                          else (R - pad, R))
                                nc.vector.memset(
                                    xtiles[j][:, z0 * H:z1 * H], 0.0)
                            nc.sync.dma_start(
                                out=xtiles[j][:, s * H:e * H],
                                in_=xwin_d[:, _OFFS[j] + s * H:
                                           _OFFS[j] + e * H])

            taps = _TAPS1
            last_pos = {}
            for i, (u, v, rh, ch, s) in enumerate(taps):
                last_pos[(rh, ch)] = i

            def evac(di, sub, ptA, ptB):
                ot = opool.tile([64, SUB * H], mybir.dt.float32, tag="ot")
                if variant == "dve4":
                    # walrus: only one non-scalar input may read PSUM per op
                    nc.vector.tensor_copy(ot[:], ptA[0:64, :])
                    nc.vector.tensor_add(ot[:], ot[:], ptA[64:128, :])
                    nc.vector.tensor_add(ot[:], ot[:], ptB[0:64, :])
                    nc.vector.tensor_add(ot[:], ot[:], ptB[64:128, :])
                elif variant == "evac1":
                    nc.vector.tensor_copy(ot[:], ptA[0:64, :])
                else:
                    # Act engine does the copy (psum read is partition-
                    # aligned); DVE does the 3 accumulating adds.  Cross-
                    # partition reads are allowed only from PSUM, and only
                    # one PSUM operand per op (walrus rules).
                    nc.scalar.activation(ot[:], ptA[0:64, :],
                                         mybir.ActivationFunctionType.Copy)
                    nc.vector.tensor_add(ot[:], ot[:], ptA[64:128, :])
                    nc.vector.tensor_add(ot[:], ot[:], ptB[0:64, :])
                    nc.vector.tensor_add(ot[:], ot[:], ptB[64:128, :])
                seg = (di * NSUB + sub) * SUB * H
                nc.sync.dma_start(out=out_d[:, seg:seg + SUB * H], in_=ot[:])

            def mm(d, sub, tap, banks, started):
                (u, v, rh, ch, s) = tap
                j, half = _VIEW_LOC[(u, v)]
                au = abs(4 - u)
                row0 = sub * SUB + d * (4 - u) + 4 * au
                sv = d * (4 - v)
                if variant == "fullw":
                    # timing-only: full-width windows (wrong numerics) to
                    # measure the cost of short-inner-run clipped APs
                    sv = 0
                wlo = max(0, -sv)
                whi = min(H, H - sv)
                rhs = xviews[j][rh * 64:(rh + 1) * 64,
                                row0:row0 + SUB,
                                wlo + sv:whi + sv]
                kh, kw = (u, v) if d <= 0 else (8 - u, 8 - v)
                kidx = kh * A + kw
                lhsT = wtr[rh * 64:(rh + 1) * 64,
                           kidx * C:(kidx + 1) * C]
                pt = banks[rh]
                if wlo == 0 and whi == H:
                    outap = pt[ch * 64:(ch + 1) * 64, :]
                else:
                    # 3D psum out AP (h stride 48, contiguous w run): HW
                    # handles it; CoreSim needs the assert patch in test.py.
                    ptv = pt[:].rearrange("p (r c) -> p r c", r=SUB, c=H)
                    outap = ptv[ch * 64:(ch + 1) * 64, 0:SUB, wlo:whi]
                key = (sub, rh, ch)
                nc.tensor.matmul(
                    outap, lhsT, rhs,
                    start=(key not in started),
                    stop=False,
                    tile_position=(rh * 64, ch * 64),
                    skip_group_check=True,
                )
                started.add(key)

            # per-d lightest ch per rh, for placing the (4,4) half-MMs
            ch44 = {}
            for dd in range(-4, 5):
                base = {(r, c): 0 for r in (0, 1) for c in (0, 1)}
                for (u, v, rh, ch, s) in taps:
                    if (u, v) == (4, 4):
                        continue
                    base[(rh, ch)] += H - abs(dd * (4 - v))
                for r in (0, 1):
                    ch44[(dd, r)] = 0 if base[(r, 0)] <= base[(r, 1)] else 1

            def mm44(d, sub, banks, started):
                # (4,4) singleton: no shift/clip for any d; split into two
                # half-width MMs, one per row-half (data duplicated on both
                # tile halves), each on that rh's lightest quadrant.
                j, _ = _VIEW_LOC[(4, 4)]
                kidx = 4 * A + 4
                row0 = sub * SUB
                for rh, c0, c1 in ((0, 0, H // 2), (1, H // 2, H)):
                    ch = ch44[(d, rh)]
                    rhs = xviews[j][rh * 64:(rh + 1) * 64,
                                    row0:row0 + SUB, c0:c1]
                    lhsT = wtr[rh * 64:(rh + 1) * 64,
                               kidx * C:(kidx + 1) * C]
                    pt = banks[rh]
                    ptv = pt[:].rearrange("p (r c) -> p r c", r=SUB, c=H)
                    outap = ptv[ch * 64:(ch + 1) * 64, 0:SUB, c0:c1]
                    key = (sub, rh, ch)
                    nc.tensor.matmul(
                        outap, lhsT, rhs,
                        start=(key not in started), stop=False,
                        tile_position=(rh * 64, ch * 64),
                        skip_group_check=True,
                    )
                    started.add(key)

            def emit_sweep():
                # concurrent tile-position matmuls must write DISJOINT psum
                # regions (same-region row-tile accumulation crashes the exec
                # unit): rh0 -> bank A, rh1 -> bank B, ch picks the partition
                # half.  Taps run sub-PAIRS per weight load: the duplicate
                # LDWEIGHTS of the second MM is removed by _dedup_ldweights,
                # halving weight-bus traffic (the previous bottleneck).
                for di in range(ND):
                    d = di - 4
                    # pass 1: subs 0,1 (two MMs per tap, shared weights)
                    bank = {
                        (0, 0): ppool.tile([128, SUB * H], mybir.dt.float32,
                                           name="ptA0", tag="ptA0"),
                        (0, 1): ppool.tile([128, SUB * H], mybir.dt.float32,
                                           name="ptB0", tag="ptB0"),
                        (1, 0): ppool.tile([128, SUB * H], mybir.dt.float32,
                                           name="ptA1", tag="ptA1"),
                        (1, 1): ppool.tile([128, SUB * H], mybir.dt.float32,
                                           name="ptB1", tag="ptB1"),
                    }
                    use_taps = taps[::2] if variant == "halftaps" else taps
                    started = set()
                    for tap in use_taps:
                        if (tap[0], tap[1]) == (4, 4):
                            mm44(d, 0, (bank[(0, 0)], bank[(0, 1)]), started)
                            mm44(d, 1, (bank[(1, 0)], bank[(1, 1)]), started)
                            continue
                        mm(d, 0, tap, (bank[(0, 0)], bank[(0, 1)]), started)
                        mm(d, 1, tap, (bank[(1, 0)], bank[(1, 1)]), started)
                    evac(di, 0, bank[(0, 0)], bank[(0, 1)])
                    evac(di, 1, bank[(1, 0)], bank[(1, 1)])
                    # pass 2: sub 2
                    pA = ppool.tile([128, SUB * H], mybir.dt.float32, tag="ptA0")
                    pB = ppool.tile([128, SUB * H], mybir.dt.float32, tag="ptB0")
                    started2 = set()
                    for tap in use_taps:
                        if (tap[0], tap[1]) == (4, 4):
                            mm44(d, 2, (pA, pB), started2)
                            continue
                        mm(d, 2, tap, (pA, pB), started2)
                    evac(di, 2, pA, pB)

            if repeat == 1:
                load_x()
                emit_sweep()
            elif variant == "loopall":
                # timing-only: input DMA repeats with the sweep, so the
                # loop slope approximates a full single-shot exec
                with tc.For_i(0, repeat, 1):
                    load_x()
                    emit_sweep()
            else:
                # timing-only: repeat the full sweep in a hardware loop
                load_x()
                with tc.For_i(0, repeat, 1):
                    emit_sweep()

    _dedup_ldweights(nc)
    nc.finalize()
    return nc


def _dedup_ldweights(nc):
    """Remove InstLdweights that reload the stationary operand already
    resident at the same tile position (sub-pair MMs share weights).  The
    PE keeps independent stationary sets per (row, col) tile group, and
    only an LDW targeting the same position clobbers one."""
    removed = kept = 0
    for bb in nc.m.functions[0].blocks:
        last = {}
        to_remove = []
        for ins in bb.instructions:
            if not str(ins.engine).endswith("PE"):
                continue
            tn = type(ins).__name__
            if tn == "InstLdweights":
                si = ins.sync_info
                has_sync = si is not None and (si.on_wait or si.on_update)
                sig = (str(ins.ins[0]), str(getattr(ins, "tile_position", None)),
                       str(getattr(ins, "perf_mode", None)))
                pos = str(getattr(ins, "tile_position", None))
                if not has_sync and last.get(pos) == sig:
                    to_remove.append(ins)
                    removed += 1
                else:
                    last[pos] = sig
                    kept += 1
            elif tn == "InstMatmult":
                continue
            else:
                last.clear()
        for ins in to_remove:
            bb.instructions.remove(ins)
    if removed:
        import logging
        logging.getLogger(__name__).info(
            "dedup_ldweights: removed %d, kept %d", removed, kept)


def get_nc(repeat=1):
    import os
    key = ("nc", repeat, os.environ.get("KVARIANT", "full"))
    if key not in _NC_CACHE:
        _NC_CACHE[key] = _build_nc(repeat)
    return _NC_CACHE[key]


def prepare_inputs(x, W):
    """Host-side packing: per-core xwin [128,F] bf16 + shared wpack."""
    x = np.asarray(x, dtype=np.float32)
    W = np.asarray(W, dtype=np.float32)
    # X5[b,u,v,ci,h,w]
    X5 = np.ascontiguousarray(
        x.reshape(B, C, H, A, H, A).transpose(0, 3, 5, 1, 2, 4)
    ).astype(BF16)

    xwins = []
    for core in range(N_CORES):
        b, hh = divmod(core, 2)
        h0 = hh * HH
        xw = np.zeros((128, _F), dtype=BF16)
        for j, (va, vb, R) in enumerate(_PAIRS):
            # the (4,4) singleton is duplicated onto the (otherwise empty)
            # second half of its tile so its matmul can be split across
            # both row-halves for quadrant load balance (same DMA bytes —
            # that half shipped zeros before).
            for half, view in ((0, va), (1, vb if vb is not None else va)):
                u, v = view
                lo = h0 - 4 * abs(4 - u)
                vs = max(0, lo)
                ve = min(H, lo + R)
                blk = X5[b, u, v, :, vs:ve, :]  # [64, ve-vs, 48]
                dst = xw[half * 64:(half + 1) * 64,
                         _OFFS[j]:_OFFS[j] + R * H].reshape(64, R, H)
                dst[:, vs - lo:ve - lo, :] = blk
        xwins.append(xw)

    # wt[ci + 64*half, (kh*9+kw)*64 + co] = W[co, ci, kh, kw], both halves
    wt1 = np.ascontiguousarray(
        W.transpose(1, 2, 3, 0).reshape(C, A * A * C)).astype(BF16)
    wtrans = np.concatenate([wt1, wt1], axis=0)
    return xwins, wtrans


def assemble_output(results):
    """results: list of 8 dicts with 'out' [64, ND*NSUB*SUB*H] fp32."""
    full = np.empty((B, C, ND, H, H), dtype=np.float32)
    for core in range(N_CORES):
        b, hh = divmod(core, 2)
        oc = np.asarray(results[core]["out"]).reshape(C, ND, HH, H)
        full[b, :, :, hh * HH:(hh + 1) * HH, :] = oc
    return full


def make_in_maps(x, W):
    xwins, wtrans = prepare_inputs(x, W)
    return [{"xwin": xwins[c], "wt": wtrans} for c in range(N_CORES)]


def kernel(x, W):
    from concourse.bass_utils import run_bass_kernel_spmd

    nc = get_nc()
    in_maps = make_in_maps(x, W)
    res = run_bass_kernel_spmd(nc, in_maps, core_ids=list(range(N_CORES)))
    return assemble_output(res.results)



# revision 25
# speedup vs baseline: 1.0656x; 1.0656x over previous
"""BuildCostVolume kernel for 8 Trainium2 NeuronCores.

Decomposition: the 9 strided dilated convs (disparities d=-4..4) reduce to
729 taps (d,u,v): cost[b,co,d,h,w] = sum_{ci,u,v} Wd[d][co,ci,u,v] *
X[b,ci,u,v, h+d(4-u), w+d(4-v)] where X is the view-decomposed light field
(X[b,ci,u,v,h,w] = x[b,ci,9h+u,9w+v], zero outside) and Wd flips (u,v) for
d>0.  Each tap is a K=64(ci) x M=64(co) matmul over spatial positions.

Sharding: core = (batch b, h-half).  Each core holds all 81 views' h-windows
(zero-padded to uniform per-u heights) resident in SBUF as 41 view-pair
tiles ([128, R*48] bf16: two views stacked on partition halves).  Taps run
as 4-way concurrent matmuls via tile_position (2 row-groups x 2 col-groups),
accumulating per (d, 8-row subchunk) into PSUM tiles keyed (sub, rh) with
the col-group picking the partition half.  Evac: the Activation engine
copies one psum quadrant-partial while DVE does the 3 accumulating adds
(cross-partition reads are only legal from PSUM; one PSUM operand per op)
— moving the copy off DVE took the measured sweep from ~136us to ~112us.
Input load ships only the valid rows of each view block (79% of bytes; the
load is HBM-bound across the 8 cores) and branches on core id (tc.If on
nc.partition_id(), which loads the condition on ALL engines — an SP-only
register deadlocks the DVE memsets) to zero the class-dependent pad rows
on the otherwise-idle DVE while the DMAs stream.  Weights are shipped once
as the 81 pre-transposed taps [ci, co] (10KB/partition, resident in SBUF;
lhsT APs index them by (kh,kw) per (d,tap)) instead of a per-d wpack —
every disparity reuses the same 81 slices, so the old layout shipped the
weight bytes 4.5x over and spent 9 DMA sync points per sweep.

The (4,4) singleton view (the only unpaired one) is duplicated onto its
tile's empty half and its matmul split into two half-width MMs, one per
row-half on the per-d lightest quadrant — without this the 41-vs-40 tap
split between row halves inflates the 4-quadrant wall by ~3.9% (~3us).

Measured dead ends (see _transcript/memory): fp8e4 DoubleRow is blocked by
the s3d3_mm_valid_dst_partition ISA check at col position 64 (DoubleRow's
128-wide stationary uses all PE columns -> only 2-way row tiling), and the
3-term error compensation needed for rel_err<2e-2 makes it net slower than
bf16.  A ch-fixed group restructure (2 psum partials/group, 4-group sets)
measured ~163us/sweep — worse than this layout.
"""

import numpy as np
import ml_dtypes

A = 9           # angular resolution
H = 48          # spatial h/w per view
C = 64          # channels (ci = co = 64)
B = 4           # batch
ND = 9          # disparities -4..4
HH = 24         # h rows per core (half)
SUB = 8         # output h rows per psum accumulation group
NSUB = HH // SUB
NSLOT = 41      # weight slots per row-half per d
N_CORES = 8

BF16 = ml_dtypes.bfloat16


def _geometry():
    """Static tap/tile geometry shared by host packing and device program."""
    pairs = []            # (viewA, viewB-or-None, R)
    # seed tiles first: their 4 views are the full-width (v=4) taps that
    # carry start=True per tile position, so their DMAs must land first.
    pairs.append(((0, 4), (8, 4), HH + 8 * 4))
    pairs.append(((1, 4), (7, 4), HH + 8 * 3))
    for v in range(A):
        for u in range(4):
            if v == 4 and u in (0, 1):
                continue
            pairs.append(((u, v), (8 - u, v), HH + 8 * (4 - u)))
    for k in range(4):
        pairs.append(((4, k), (4, k + 5), HH))
    pairs.append(((4, 4), None, HH))

    view_loc = {}
    offs = []
    off = 0
    for j, (va, vb, R) in enumerate(pairs):
        view_loc[va] = (j, 0)
        if vb is not None:
            view_loc[vb] = (j, 1)
        offs.append(off)
        off += R * H
    F = off

    # one tap order for every d: tile order (= DMA arrival order), seeds
    # first covering the 4 (rh, ch) positions, then ch alternating per rh
    # so consecutive taps hit different tile positions.
    taps = []
    ch_ctr = {0: 0, 1: 0}
    slot_ctr = {0: 0, 1: 0}
    for j, (va, vb, R) in enumerate(pairs):
        for half, view in ((0, va), (1, vb)):
            if view is None:
                continue
            u, v = view
            rh = half
            ch = ch_ctr[rh] % 2
            ch_ctr[rh] += 1
            s = slot_ctr[rh]
            slot_ctr[rh] += 1
            taps.append((u, v, rh, ch, s))
    assert len(taps) == 81
    # seeds sanity: first 4 taps cover all 4 positions with v=4 views
    seed_pos = {(rh, ch) for (u, v, rh, ch, s) in taps[:4]}
    assert len(seed_pos) == 4 and all(v == 4 for (u, v, _, _, _) in taps[:4])

    return pairs, view_loc, offs, F, taps


_PAIRS, _VIEW_LOC, _OFFS, _F, _TAPS1 = _geometry()
_TAPS = [_TAPS1] * ND  # same order for every d (kept for prepare_inputs)
_NC_CACHE = {}


def _build_nc(repeat=1):
    import os
    import concourse.bacc as bacc
    import concourse.mybir as mybir
    import concourse.tile as tile

    variant = os.environ.get("KVARIANT", "full")  # timing experiments only

    nc = bacc.Bacc(None, target_bir_lowering=False)
    xwin_d = nc.dram_tensor("xwin", [128, _F], mybir.dt.bfloat16,
                            kind="ExternalInput")
    # 81 pre-transposed W taps [ci, co], duplicated on both partition
    # halves: every (d, tap) weight block is one of these slices, so no
    # per-d wpack staging (saves 5.3MB/core of the HBM-bound input and 9
    # DMA sync points per sweep).
    wt_d = nc.dram_tensor("wt", [128, A * A * C], mybir.dt.bfloat16,
                          kind="ExternalInput")
    out_d = nc.dram_tensor("out", [C, ND * NSUB * SUB * H], mybir.dt.float32,
                           kind="ExternalOutput")

    with tile.TileContext(nc) as tc:
        with tc.tile_pool(name="xw", bufs=1) as xpool, \
             tc.tile_pool(name="wp", bufs=1) as wpool, \
             tc.tile_pool(name="ps", bufs=2, space="PSUM") as ppool, \
             tc.tile_pool(name="ob", bufs=4) as opool:

            # resident weight taps; DMA first so the seed MMs aren't blocked
            wtr = wpool.tile([128, A * A * C], mybir.dt.bfloat16, tag="wtr")
            nc.sync.dma_start(out=wtr[:], in_=wt_d[:])

            xtiles = []
            xviews = []
            for j, (va, vb, R) in enumerate(_PAIRS):
                t = xpool.tile([128, R * H], mybir.dt.bfloat16, tag=f"x{j}")
                xtiles.append(t)
                # h-major [p, r, c]: contiguous 48-elem inner runs stream at
                # full PE rate (short strided inner runs are ~2.5x slower)
                xviews.append(t[:].rearrange("p (r c) -> p r c", r=R, c=H))

            def load_x():
                # Valid-rows-only input: each view block [R=24+8au, 48] has
                # 4au zero rows (top for h-half 0 cores, bottom for h-half 1)
                # that the host packs but we never ship — DVE memsets them
                # while the DMAs stream the remaining 79%.  Both views of a
                # pair share au, so pad ranges are tile-uniform.  The pad
                # side depends on the core's h-half -> branch on core id.
                pid = nc.partition_id()
                for cls in (0, 1):
                    with tc.If(pid % 2 == cls):
                        for j, (va, vb, R) in enumerate(_PAIRS):
                            pad = ((R - HH) // 8) * 4  # 4*au
                            s, e = (pad, R) if cls == 0 else (0, R - pad)
                            if pad:
                                z0, z1 = ((0, pad) if cls == 0
                                          else (R - pad, R))
                                nc.vector.memset(
                                    xtiles[j][:, z0 * H:z1 * H], 0.0)
                            nc.sync.dma_start(
                                out=xtiles[j][:, s * H:e * H],
                                in_=xwin_d[:, _OFFS[j] + s * H:
                                           _OFFS[j] + e * H])

            taps = _TAPS1
            last_pos = {}
            for i, (u, v, rh, ch, s) in enumerate(taps):
                last_pos[(rh, ch)] = i

            def evac(di, sub, ptA, ptB):
                ot = opool.tile([64, SUB * H], mybir.dt.float32, tag="ot")
                if variant == "dve4":
                    # walrus: only one non-scalar input may read PSUM per op
                    nc.vector.tensor_copy(ot[:], ptA[0:64, :])
                    nc.vector.tensor_add(ot[:], ot[:], ptA[64:128, :])
                    nc.vector.tensor_add(ot[:], ot[:], ptB[0:64, :])
                    nc.vector.tensor_add(ot[:], ot[:], ptB[64:128, :])
                elif variant == "evac1":
                    nc.vector.tensor_copy(ot[:], ptA[0:64, :])
                else:
                    # Act engine does the copy (psum read is partition-
                    # aligned); DVE does the 3 accumulating adds.  Cross-
                    # partition reads are allowed only from PSUM, and only
                    # one PSUM operand per op (walrus rules).
                    nc.scalar.activation(ot[:], ptA[0:64, :],
                                         mybir.ActivationFunctionType.Copy)
                    nc.vector.tensor_add(ot[:], ot[:], ptA[64:128, :])
                    nc.vector.tensor_add(ot[:], ot[:], ptB[0:64, :])
                    nc.vector.tensor_add(ot[:], ot[:], ptB[64:128, :])
                seg = (di * NSUB + sub) * SUB * H
                nc.sync.dma_start(out=out_d[:, seg:seg + SUB * H], in_=ot[:])

            def mm(d, sub, tap, banks, started):
                (u, v, rh, ch, s) = tap
                j, half = _VIEW_LOC[(u, v)]
                au = abs(4 - u)
                row0 = sub * SUB + d * (4 - u) + 4 * au
                sv = d * (4 - v)
                if variant == "fullw":
                    # timing-only: full-width windows (wrong numerics) to
                    # measure the cost of short-inner-run clipped APs
                    sv = 0
                wlo = max(0, -sv)
                whi = min(H, H - sv)
                rhs = xviews[j][rh * 64:(rh + 1) * 64,
                                row0:row0 + SUB,
                                wlo + sv:whi + sv]
                kh, kw = (u, v) if d <= 0 else (8 - u, 8 - v)
                kidx = kh * A + kw
                lhsT = wtr[rh * 64:(rh + 1) * 64,
                           kidx * C:(kidx + 1) * C]
                pt = banks[rh]
                if wlo == 0 and whi == H:
                    outap = pt[ch * 64:(ch + 1) * 64, :]
                else:
                    # 3D psum out AP (h stride 48, contiguous w run): HW
                    # handles it; CoreSim needs the assert patch in test.py.
                    ptv = pt[:].rearrange("p (r c) -> p r c", r=SUB, c=H)
                    outap = ptv[ch * 64:(ch + 1) * 64, 0:SUB, wlo:whi]
                key = (sub, rh, ch)
                nc.tensor.matmul(
                    outap, lhsT, rhs,
                    start=(key not in started),
                    stop=False,
                    tile_position=(rh * 64, ch * 64),
                    skip_group_check=True,
                )
                started.add(key)

            # per-d lightest ch per rh, for placing the (4,4) half-MMs
            ch44 = {}
            for dd in range(-4, 5):
                base = {(r, c): 0 for r in (0, 1) for c in (0, 1)}
                for (u, v, rh, ch, s) in taps:
                    if (u, v) == (4, 4):
                        continue
                    base[(rh, ch)] += H - abs(dd * (4 - v))
                for r in (0, 1):
                    ch44[(dd, r)] = 0 if base[(r, 0)] <= base[(r, 1)] else 1

            def mm44(d, sub, banks, started):
                # (4,4) singleton: no shift/clip for any d; split into two
                # half-width MMs, one per row-half (data duplicated on both
                # tile halves), each on that rh's lightest quadrant.
                j, _ = _VIEW_LOC[(4, 4)]
                kidx = 4 * A + 4
                row0 = sub * SUB
                for rh, c0, c1 in ((0, 0, H // 2), (1, H // 2, H)):
                    ch = ch44[(d, rh)]
                    rhs = xviews[j][rh * 64:(rh + 1) * 64,
                                    row0:row0 + SUB, c0:c1]
                    lhsT = wtr[rh * 64:(rh + 1) * 64,
                               kidx * C:(kidx + 1) * C]
                    pt = banks[rh]
                    ptv = pt[:].rearrange("p (r c) -> p r c", r=SUB, c=H)
                    outap = ptv[ch * 64:(ch + 1) * 64, 0:SUB, c0:c1]
                    key = (sub, rh, ch)
                    nc.tensor.matmul(
                        outap, lhsT, rhs,
                        start=(key not in started), stop=False,
                        tile_position=(rh * 64, ch * 64),
                        skip_group_check=True,
                    )
                    started.add(key)

            def alloc_banks():
                return {
                    (0, 0): ppool.tile([128, SUB * H], mybir.dt.float32,
                                       name="bA0", tag="ptA0"),
                    (0, 1): ppool.tile([128, SUB * H], mybir.dt.float32,
                                       name="bB0", tag="ptB0"),
                    (1, 0): ppool.tile([128, SUB * H], mybir.dt.float32,
                                       name="bA1", tag="ptA1"),
                    (1, 1): ppool.tile([128, SUB * H], mybir.dt.float32,
                                       name="bB1", tag="ptB1"),
                }

            def pass1(d, bank, started):
                for tap in taps:
                    if (tap[0], tap[1]) == (4, 4):
                        mm44(d, 0, (bank[(0, 0)], bank[(0, 1)]), started)
                        mm44(d, 1, (bank[(1, 0)], bank[(1, 1)]), started)
                        continue
                    mm(d, 0, tap, (bank[(0, 0)], bank[(0, 1)]), started)
                    mm(d, 1, tap, (bank[(1, 0)], bank[(1, 1)]), started)

            def emit_sweep():
                # concurrent tile-position matmuls must write DISJOINT psum
                # regions: rh0 -> bank A, rh1 -> bank B, ch picks the
                # partition half.  The first TWO disparities' pass-1 walks
                # are interleaved per tap so ~16us of compute (instead of
                # ~8) overlaps the HBM-bound input stream; the tag rings
                # (bufs=2) hold both generations.  LDWEIGHTS of same-
                # weight MM pairs are removed by _dedup_ldweights.
                bankP = alloc_banks()
                bankQ = alloc_banks()
                stP, stQ = set(), set()
                for tap in taps:
                    if (tap[0], tap[1]) == (4, 4):
                        mm44(-4, 0, (bankP[(0, 0)], bankP[(0, 1)]), stP)
                        mm44(-4, 1, (bankP[(1, 0)], bankP[(1, 1)]), stP)
                        mm44(-3, 0, (bankQ[(0, 0)], bankQ[(0, 1)]), stQ)
                        mm44(-3, 1, (bankQ[(1, 0)], bankQ[(1, 1)]), stQ)
                        continue
                    mm(-4, 0, tap, (bankP[(0, 0)], bankP[(0, 1)]), stP)
                    mm(-4, 1, tap, (bankP[(1, 0)], bankP[(1, 1)]), stP)
                    mm(-3, 0, tap, (bankQ[(0, 0)], bankQ[(0, 1)]), stQ)
                    mm(-3, 1, tap, (bankQ[(1, 0)], bankQ[(1, 1)]), stQ)
                for dd, bk in ((-4, bankP), (-3, bankQ)):
                    evac(dd + 4, 0, bk[(0, 0)], bk[(0, 1)])
                    evac(dd + 4, 1, bk[(1, 0)], bk[(1, 1)])

                for di in range(ND):
                    d = di - 4
                    if d > -3:
                        # pass 1: subs 0,1 (two MMs per tap, shared
                        # weights); d=-4,-3 already ran theirs above
                        bank = alloc_banks()
                        started = set()
                        pass1(d, bank, started)
                        evac(di, 0, bank[(0, 0)], bank[(0, 1)])
                        evac(di, 1, bank[(1, 0)], bank[(1, 1)])
                    # pass 2: sub 2
                    pA = ppool.tile([128, SUB * H], mybir.dt.float32,
                                    name="pA", tag="ptA0")
                    pB = ppool.tile([128, SUB * H], mybir.dt.float32,
                                    name="pB", tag="ptB0")
                    started2 = set()
                    for tap in taps:
                        if (tap[0], tap[1]) == (4, 4):
                            mm44(d, 2, (pA, pB), started2)
                            continue
                        mm(d, 2, tap, (pA, pB), started2)
                    evac(di, 2, pA, pB)

            if repeat == 1:
                load_x()
                emit_sweep()
            elif variant == "loopall":
                # timing-only: input DMA repeats with the sweep, so the
                # loop slope approximates a full single-shot exec
                with tc.For_i(0, repeat, 1):
                    load_x()
                    emit_sweep()
            else:
                # timing-only: repeat the full sweep in a hardware loop
                load_x()
                with tc.For_i(0, repeat, 1):
                    emit_sweep()

    _dedup_ldweights(nc)
    nc.finalize()
    return nc


def _dedup_ldweights(nc):
    """Remove InstLdweights that reload the stationary operand already
    resident at the same tile position (sub-pair MMs share weights).  The
    PE keeps independent stationary sets per (row, col) tile group, and
    only an LDW targeting the same position clobbers one."""
    removed = kept = 0
    for bb in nc.m.functions[0].blocks:
        last = {}
        to_remove = []
        for ins in bb.instructions:
            if not str(ins.engine).endswith("PE"):
                continue
            tn = type(ins).__name__
            if tn == "InstLdweights":
                si = ins.sync_info
                has_sync = si is not None and (si.on_wait or si.on_update)
                sig = (str(ins.ins[0]), str(getattr(ins, "tile_position", None)),
                       str(getattr(ins, "perf_mode", None)))
                pos = str(getattr(ins, "tile_position", None))
                if not has_sync and last.get(pos) == sig:
                    to_remove.append(ins)
                    removed += 1
                else:
                    last[pos] = sig
                    kept += 1
            elif tn == "InstMatmult":
                continue
            else:
                last.clear()
        for ins in to_remove:
            bb.instructions.remove(ins)
    if removed:
        import logging
        logging.getLogger(__name__).info(
            "dedup_ldweights: removed %d, kept %d", removed, kept)


def get_nc(repeat=1):
    import os
    key = ("nc", repeat, os.environ.get("KVARIANT", "full"))
    if key not in _NC_CACHE:
        _NC_CACHE[key] = _build_nc(repeat)
    return _NC_CACHE[key]


def prepare_inputs(x, W):
    """Host-side packing: per-core xwin [128,F] bf16 + shared wpack."""
    x = np.asarray(x, dtype=np.float32)
    W = np.asarray(W, dtype=np.float32)
    # X5[b,u,v,ci,h,w]
    X5 = np.ascontiguousarray(
        x.reshape(B, C, H, A, H, A).transpose(0, 3, 5, 1, 2, 4)
    ).astype(BF16)

    xwins = []
    for core in range(N_CORES):
        b, hh = divmod(core, 2)
        h0 = hh * HH
        xw = np.zeros((128, _F), dtype=BF16)
        for j, (va, vb, R) in enumerate(_PAIRS):
            # the (4,4) singleton is duplicated onto the (otherwise empty)
            # second half of its tile so its matmul can be split across
            # both row-halves for quadrant load balance (same DMA bytes —
            # that half shipped zeros before).
            for half, view in ((0, va), (1, vb if vb is not None else va)):
                u, v = view
                lo = h0 - 4 * abs(4 - u)
                vs = max(0, lo)
                ve = min(H, lo + R)
                blk = X5[b, u, v, :, vs:ve, :]  # [64, ve-vs, 48]
                dst = xw[half * 64:(half + 1) * 64,
                         _OFFS[j]:_OFFS[j] + R * H].reshape(64, R, H)
                dst[:, vs - lo:ve - lo, :] = blk
        xwins.append(xw)

    # wt[ci + 64*half, (kh*9+kw)*64 + co] = W[co, ci, kh, kw], both halves
    wt1 = np.ascontiguousarray(
        W.transpose(1, 2, 3, 0).reshape(C, A * A * C)).astype(BF16)
    wtrans = np.concatenate([wt1, wt1], axis=0)
    return xwins, wtrans


def assemble_output(results):
    """results: list of 8 dicts with 'out' [64, ND*NSUB*SUB*H] fp32."""
    full = np.empty((B, C, ND, H, H), dtype=np.float32)
    for core in range(N_CORES):
        b, hh = divmod(core, 2)
        oc = np.asarray(results[core]["out"]).reshape(C, ND, HH, H)
        full[b, :, :, hh * HH:(hh + 1) * HH, :] = oc
    return full


def make_in_maps(x, W):
    xwins, wtrans = prepare_inputs(x, W)
    return [{"xwin": xwins[c], "wt": wtrans} for c in range(N_CORES)]


def kernel(x, W):
    from concourse.bass_utils import run_bass_kernel_spmd

    nc = get_nc()
    in_maps = make_in_maps(x, W)
    res = run_bass_kernel_spmd(nc, in_maps, core_ids=list(range(N_CORES)))
    return assemble_output(res.results)

